# revision 33
# baseline (speedup 1.0000x reference)
"""Trainium2 Bass kernel for nn_CIntegration_3487513444382 (embedding_lookup).

Computation (per token): ct = concat(onehot(rgap,32), onehot(sgap,32),
onehot(pcount,32)); out = concat(vt * (ct @ W.T), ct).

Strategy: pure data parallel over the batch dim (64 -> 8 per core), with
all device-side tensors in E-major ("transposed") layout so the one-hot
is built exactly once: ct_T [96, ntok] doubles as the matmul moving
operand AND the output tail. The host ships vt_T bf16 plus the offset
indices pre-replicated across the 96 one-hot rows as uint8 (a pure
layout transform of the int inputs); the device builds the exact
one-hot with DVE compares vs the partition iota (no PE broadcast pass),
applies W as a stationary bf16 operand into fp32 PSUM, gates with vt,
and streams theta_T bf16 + ct_T fp8 out. PSUM drain is balanced
between ACT (copy to bf16 SBUF, cheap all-SBUF DVE gate after) and
direct-from-PSUM DVE gates so no engine is the wall. DMA issue is
spread over the three DMA-capable engines (Sync: bcast loads early +
theta stores, Pool: vt loads, ACT: wt load + ct stores). The host
transposes and upcasts to fp32 (wall-clock only, not device time).
bf16/fp8/u8 I/O halves HBM traffic vs fp32 (~10 MB/core); end-to-end
error ~6e-3 of output scale vs the 2e-2 gate.
"""
import numpy as np

import concourse.bass as bass
import concourse.tile as tile
from concourse import bacc, mybir
from concourse.bass_utils import run_bass_kernel_spmd

F32 = mybir.dt.float32
BF16 = mybir.dt.bfloat16
FP8 = mybir.dt.float8e4
U8 = mybir.dt.uint8

N_CORES = 8
B, S, E = 64, 1024, 256
BPC = B // N_CORES          # 8 batches per core
NTOK = BPC * S              # 8192 tokens per core
NTOT = 96                   # one-hot width
NH = E // 128               # 2 E-halves of 128 partitions
TQ = 4096                   # tokens per vt load DMA
# graduated compute blocks: small lead blocks cut the front-end ramp
# (first matmul train starts right after the first small compare),
# full-size blocks amortize per-op overhead in steady state
BLOCKS = [1024, 1024, 2048, 2048, 2048]
NB = len(BLOCKS)
MMN = 512                   # moving cols per matmul (one PSUM bank out)

_NC = None


def _build_nc():
    nc = bacc.Bacc("TRN2", target_bir_lowering=False, debug=False,
                   num_devices=N_CORES)
    vt_t = nc.dram_tensor("vt_t", [E, NTOK], BF16, kind="ExternalInput")
    bcast = nc.dram_tensor("bcast", [NTOT, NTOK], U8, kind="ExternalInput")
    wt = nc.dram_tensor("wt", [NTOT, E], BF16, kind="ExternalInput")
    theta_t = nc.dram_tensor("theta_t", [E, NTOK], BF16,
                             kind="ExternalOutput")
    ct_t = nc.dram_tensor("ct_t", [NTOT, NTOK], FP8, kind="ExternalOutput")

    with tile.TileContext(nc) as tc:
        with (
            tc.tile_pool(name="const", bufs=1) as const,
            tc.tile_pool(name="vtp", bufs=NTOK // TQ * NH) as vtp,
            tc.tile_pool(name="thp", bufs=8) as thp,
            tc.tile_pool(name="ctp", bufs=NB) as ctp,
            tc.tile_pool(name="bcp", bufs=NB) as bcp,
            tc.tile_pool(name="ccp", bufs=3) as ccp,
            tc.tile_pool(name="ps_m", bufs=2, space="PSUM") as ps_m,
        ):
            vt_view = vt_t.ap().rearrange("(h p) t -> h p t", h=NH)
            th_view = theta_t.ap().rearrange("(h p) t -> h p t", h=NH)

            # all loads share the Pool sw queue: FIFO order IS the
            # priority order (cross-queue traffic is fair-shared, so a
            # separate queue would not give the bcast chunks priority)
            wt_sb = const.tile([NTOT, E], BF16)
            nc.scalar.dma_start(wt_sb[:], wt.ap())
            iota_col = const.tile([NTOT, 1], F32)
            bc_sb, vt_sb = {}, {}
            OFF = [sum(BLOCKS[:i]) for i in range(NB)]

            def load_bc(b):
                bc_sb[b] = bcp.tile([NTOT, BLOCKS[b]], U8, name="bc_in",
                                    tag="bc_in", padded_shape=[NTOT, 2048])
                nc.gpsimd.dma_start(
                    bc_sb[b][:], bcast.ap()[:, OFF[b]:OFF[b] + BLOCKS[b]])

            def load_vt(h, q):
                vt_sb[h, q] = vtp.tile([128, TQ], BF16, name="vt_in",
                                       tag="vt_in")
                nc.gpsimd.dma_start(
                    vt_sb[h, q][:], vt_view[h, :, q * TQ:(q + 1) * TQ])

            load_bc(0); load_bc(1)
            # iota sits between load issues: cmp 0 needs it only after
            # bcast chunk 0 lands, and issuing loads first buys ~0.5us
            nc.gpsimd.iota(iota_col[:], [[0, 1]], channel_multiplier=1,
                           allow_small_or_imprecise_dtypes=True)
            load_bc(2); load_vt(0, 0)
            load_bc(3); load_bc(4); load_vt(1, 0)
            load_vt(0, 1); load_vt(1, 1)

            def cmp(b):
                # exact one-hot: integer compare vs the partition index
                ct = ctp.tile([NTOT, BLOCKS[b]], FP8, name="ct", tag="ct",
                              padded_shape=[NTOT, 2048])
                nc.vector.tensor_scalar(
                    ct[:], bc_sb[b][:], iota_col[:, 0:1], None,
                    mybir.AluOpType.is_equal,
                )
                # ct goes out on the ACT hw ring so theta stores are not
                # queued behind it on the Sync ring
                nc.scalar.dma_start(
                    ct_t.ap()[:, OFF[b]:OFF[b] + BLOCKS[b]], ct[:])
                return ct

            ct_sb = {0: cmp(0), 1: cmp(1)}
            for b in range(NB):
                c0, CBb = OFF[b], BLOCKS[b]
                q, qo = divmod(c0, TQ)
                if b + 2 < NB:
                    ct_sb[b + 2] = cmp(b + 2)
                for h in range(NH):
                    th_sb = thp.tile([128, CBb], BF16, tag="th",
                                     padded_shape=[128, 2048])
                    mm_ps = ps_m.tile([128, CBb], F32, tag="mm",
                                      padded_shape=[128, 2048])
                    for j in range(CBb // MMN):
                        # Cct.T: stationary W half, moving one-hot
                        nc.tensor.matmul(
                            mm_ps[:, j * MMN:(j + 1) * MMN],
                            wt_sb[:, h * 128:(h + 1) * 128],
                            ct_sb[b][:, j * MMN:(j + 1) * MMN],
                            start=True, stop=True,
                        )
                    vt_blk = vt_sb[h, q][:, qo:qo + CBb]
                    if h == 1 and b == NB - 1:
                        # endgame: halve the gate+store so the final
                        # store starts ~1us earlier
                        for e in range(2):
                            HB = CBb // 2
                            nc.vector.tensor_tensor(
                                th_sb[:, e * HB:(e + 1) * HB],
                                vt_blk[:, e * HB:(e + 1) * HB],
                                mm_ps[:, e * HB:(e + 1) * HB],
                                mybir.AluOpType.mult,
                            )
                            nc.sync.dma_start(
                                th_view[h, :, c0 + e * HB:
                                        c0 + (e + 1) * HB],
                                th_sb[:, e * HB:(e + 1) * HB])
                        continue
                    if h == 1:
                        # gate straight from PSUM on DVE
                        nc.vector.tensor_tensor(
                            th_sb[:], vt_blk, mm_ps[:],
                            mybir.AluOpType.mult,
                        )
                    else:
                        # drain PSUM on the idle ACT engine, then the
                        # gate runs all-SBUF bf16 on DVE; alternating
                        # drain engines with PE's h0/h1 fill alternation
                        # keeps PSUM from backing up
                        cc_sb = ccp.tile([128, CBb], BF16, tag="cc",
                                         padded_shape=[128, 2048])
                        nc.scalar.copy(cc_sb[:], mm_ps[:])
                        nc.vector.tensor_tensor(
                            th_sb[:], vt_blk, cc_sb[:],
                            mybir.AluOpType.mult,
                        )
                    nc.sync.dma_start(
                        th_view[h, :, c0:c0 + CBb], th_sb[:])

    nc.compile()
    return nc


def _get_nc():
    global _NC
    if _NC is None:
        _NC = _build_nc()
    return _NC


def _host_prep(vt, rgap, sgap, pcount, W):
    import ml_dtypes
    bf16 = ml_dtypes.bfloat16
    vt = np.asarray(vt, dtype=np.float32)
    rgap = np.asarray(rgap)
    sgap = np.asarray(sgap)
    pcount = np.asarray(pcount)
    W = np.asarray(W, dtype=np.float32)
    wt = np.ascontiguousarray(W.T).astype(bf16)     # [96, 256]
    in_maps = []
    for m in range(N_CORES):
        sl = slice(m * BPC, (m + 1) * BPC)
        vt_T = np.ascontiguousarray(
            vt[sl].reshape(NTOK, E).T).astype(bf16)  # [256, 8192]
        idxs = np.stack(
            [rgap[sl].reshape(NTOK),
             sgap[sl].reshape(NTOK) + 32,
             pcount[sl].reshape(NTOK) + 64], axis=0
        ).astype(np.uint8)                           # [3, 8192]
        bcast = np.repeat(idxs, NTOT // 3, axis=0)   # [96, 8192]
        in_maps.append({"vt_t": vt_T, "bcast": bcast, "wt": wt})
    return in_maps


def kernel(vt, rgap, sgap, pcount, W, _trace=False, _tmpdir=None):
    nc = _get_nc()
    in_maps = _host_prep(vt, rgap, sgap, pcount, W)
    res = run_bass_kernel_spmd(
        nc, in_maps, list(range(N_CORES)),
        trace=_trace, **({"tmpdir": _tmpdir} if _tmpdir else {}),
    )
    full = np.empty((B, S, E + NTOT), dtype=np.float32)
    for m in range(N_CORES):
        sl = slice(m * BPC, (m + 1) * BPC)
        theta = np.asarray(res.results[m]["theta_t"]).astype(np.float32)
        ct = np.asarray(res.results[m]["ct_t"]).astype(np.float32)
        full[sl, :, :E] = theta.T.reshape(BPC, S, E)
        full[sl, :, E:] = ct.T.reshape(BPC, S, NTOT)
    if _trace:
        return full, res
    return full


# revision 36
# speedup vs baseline: 1.0409x; 1.0409x over previous
"""Trainium2 Bass kernel for nn_CIntegration_3487513444382 (embedding_lookup).

Computation (per token): ct = concat(onehot(rgap,32), onehot(sgap,32),
onehot(pcount,32)); out = concat(vt * (ct @ W.T), ct).

Strategy: pure data parallel over the batch dim (64 -> 8 per core), with
all device-side tensors in E-major ("transposed") layout so the one-hot
is built exactly once: ct_T [96, ntok] doubles as the matmul moving
operand AND the output tail. The host ships vt_T bf16 plus the offset
indices pre-replicated across the 96 one-hot rows as uint8 (a pure
layout transform of the int inputs); the device builds the exact
one-hot with DVE compares vs the partition iota (no PE broadcast pass),
applies W as a stationary bf16 operand into fp32 PSUM, gates with vt,
and streams theta_T bf16 + ct_T fp8 out. PSUM drain is balanced
between ACT (copy to bf16 SBUF, cheap all-SBUF DVE gate after) and
direct-from-PSUM DVE gates so no engine is the wall. DMA issue is
spread over the three DMA-capable engines (Sync: bcast loads early +
theta stores, Pool: vt loads, ACT: wt load + ct stores). The host
transposes and upcasts to fp32 (wall-clock only, not device time).
bf16/fp8/u8 I/O halves HBM traffic vs fp32 (~10 MB/core); end-to-end
error ~6e-3 of output scale vs the 2e-2 gate.
"""
import numpy as np

import concourse.bass as bass
import concourse.tile as tile
from concourse import bacc, mybir
from concourse.bass_utils import run_bass_kernel_spmd

F32 = mybir.dt.float32
BF16 = mybir.dt.bfloat16
FP8 = mybir.dt.float8e4
U8 = mybir.dt.uint8

N_CORES = 8
B, S, E = 64, 1024, 256
BPC = B // N_CORES          # 8 batches per core
NTOK = BPC * S              # 8192 tokens per core
NTOT = 96                   # one-hot width
NH = E // 128               # 2 E-halves of 128 partitions
TQ = 4096                   # tokens per vt load DMA
BLOCKS = [2048, 2048, 2048, 2048]   # tokens per compute block (4 banks)
NB = len(BLOCKS)
MMN = 512                   # moving cols per matmul (one PSUM bank out)

_NC = None


def _build_nc():
    nc = bacc.Bacc("TRN2", target_bir_lowering=False, debug=False,
                   num_devices=N_CORES)
    vt_t = nc.dram_tensor("vt_t", [E, NTOK], BF16, kind="ExternalInput")
    bcast = nc.dram_tensor("bcast", [NTOT, NTOK], U8, kind="ExternalInput")
    wt = nc.dram_tensor("wt", [NTOT, E], BF16, kind="ExternalInput")
    theta_t = nc.dram_tensor("theta_t", [E, NTOK], BF16,
                             kind="ExternalOutput")
    ct_t = nc.dram_tensor("ct_t", [NTOT, NTOK], FP8, kind="ExternalOutput")

    with tile.TileContext(nc) as tc:
        with (
            tc.tile_pool(name="const", bufs=1) as const,
            tc.tile_pool(name="vtp", bufs=NTOK // TQ * NH) as vtp,
            tc.tile_pool(name="thp", bufs=8) as thp,
            tc.tile_pool(name="ctp", bufs=NB) as ctp,
            tc.tile_pool(name="bcp", bufs=NB) as bcp,
            tc.tile_pool(name="ccp", bufs=3) as ccp,
            tc.tile_pool(name="ps_m", bufs=2, space="PSUM") as ps_m,
        ):
            vt_view = vt_t.ap().rearrange("(h p) t -> h p t", h=NH)
            th_view = theta_t.ap().rearrange("(h p) t -> h p t", h=NH)

            # all loads share the Pool sw queue: FIFO order IS the
            # priority order (cross-queue traffic is fair-shared, so a
            # separate queue would not give the bcast chunks priority)
            wt_sb = const.tile([NTOT, E], BF16)
            nc.scalar.dma_start(wt_sb[:], wt.ap())
            iota_col = const.tile([NTOT, 1], F32)
            bc_sb, vt_sb = {}, {}
            OFF = [sum(BLOCKS[:i]) for i in range(NB)]

            def load_bc(b):
                bc_sb[b] = bcp.tile([NTOT, BLOCKS[b]], U8, name="bc_in",
                                    tag="bc_in", padded_shape=[NTOT, 2048])
                nc.gpsimd.dma_start(
                    bc_sb[b][:], bcast.ap()[:, OFF[b]:OFF[b] + BLOCKS[b]])

            def load_vt(h, q):
                vt_sb[h, q] = vtp.tile([128, TQ], BF16, name="vt_in",
                                       tag="vt_in")
                nc.gpsimd.dma_start(
                    vt_sb[h, q][:], vt_view[h, :, q * TQ:(q + 1) * TQ])

            load_bc(0); load_bc(1)
            # iota sits between load issues: cmp 0 needs it only after
            # bcast chunk 0 lands, and issuing loads first buys ~0.5us
            nc.gpsimd.iota(iota_col[:], [[0, 1]], channel_multiplier=1,
                           allow_small_or_imprecise_dtypes=True)
            load_vt(0, 0)
            load_bc(2); load_bc(3); load_vt(1, 0)
            load_vt(0, 1); load_vt(1, 1)

            def cmp(b):
                # exact one-hot: integer compare vs the partition index
                ct = ctp.tile([NTOT, BLOCKS[b]], FP8, name="ct", tag="ct",
                              padded_shape=[NTOT, 2048])
                nc.vector.tensor_scalar(
                    ct[:], bc_sb[b][:], iota_col[:, 0:1], None,
                    mybir.AluOpType.is_equal,
                )
                nc.sync.dma_start(
                    ct_t.ap()[:, OFF[b]:OFF[b] + BLOCKS[b]], ct[:])
                return ct

            ct_sb = {0: cmp(0), 1: cmp(1)}
            for b in range(NB):
                c0, CBb = OFF[b], BLOCKS[b]
                q, qo = divmod(c0, TQ)
                if b + 2 < NB:
                    ct_sb[b + 2] = cmp(b + 2)
                for h in range(NH):
                    th_sb = thp.tile([128, CBb], BF16, tag="th",
                                     padded_shape=[128, 2048])
                    mm_ps = ps_m.tile([128, CBb], F32, tag="mm",
                                      padded_shape=[128, 2048])
                    for j in range(CBb // MMN):
                        # Cct.T: stationary W half, moving one-hot
                        nc.tensor.matmul(
                            mm_ps[:, j * MMN:(j + 1) * MMN],
                            wt_sb[:, h * 128:(h + 1) * 128],
                            ct_sb[b][:, j * MMN:(j + 1) * MMN],
                            start=True, stop=True,
                        )
                    vt_blk = vt_sb[h, q][:, qo:qo + CBb]
                    if h == 1 and b == NB - 1:
                        # endgame: halve the gate+store so the final
                        # store starts ~1us earlier
                        for e in range(2):
                            HB = CBb // 2
                            nc.vector.tensor_tensor(
                                th_sb[:, e * HB:(e + 1) * HB],
                                vt_blk[:, e * HB:(e + 1) * HB],
                                mm_ps[:, e * HB:(e + 1) * HB],
                                mybir.AluOpType.mult,
                            )
                            nc.sync.dma_start(
                                th_view[h, :, c0 + e * HB:
                                        c0 + (e + 1) * HB],
                                th_sb[:, e * HB:(e + 1) * HB])
                        continue
                    if h == 1:
                        # gate straight from PSUM on DVE
                        nc.vector.tensor_tensor(
                            th_sb[:], vt_blk, mm_ps[:],
                            mybir.AluOpType.mult,
                        )
                    else:
                        # drain PSUM on the idle ACT engine, then the
                        # gate runs all-SBUF bf16 on DVE; alternating
                        # drain engines with PE's h0/h1 fill alternation
                        # keeps PSUM from backing up
                        cc_sb = ccp.tile([128, CBb], BF16, tag="cc",
                                         padded_shape=[128, 2048])
                        nc.scalar.copy(cc_sb[:], mm_ps[:])
                        nc.vector.tensor_tensor(
                            th_sb[:], vt_blk, cc_sb[:],
                            mybir.AluOpType.mult,
                        )
                    nc.sync.dma_start(
                        th_view[h, :, c0:c0 + CBb], th_sb[:])

    nc.compile()
    return nc


def _get_nc():
    global _NC
    if _NC is None:
        _NC = _build_nc()
    return _NC


def _host_prep(vt, rgap, sgap, pcount, W):
    import ml_dtypes
    bf16 = ml_dtypes.bfloat16
    vt = np.asarray(vt, dtype=np.float32)
    rgap = np.asarray(rgap)
    sgap = np.asarray(sgap)
    pcount = np.asarray(pcount)
    W = np.asarray(W, dtype=np.float32)
    wt = np.ascontiguousarray(W.T).astype(bf16)     # [96, 256]
    in_maps = []
    for m in range(N_CORES):
        sl = slice(m * BPC, (m + 1) * BPC)
        vt_T = np.ascontiguousarray(
            vt[sl].reshape(NTOK, E).T).astype(bf16)  # [256, 8192]
        idxs = np.stack(
            [rgap[sl].reshape(NTOK),
             sgap[sl].reshape(NTOK) + 32,
             pcount[sl].reshape(NTOK) + 64], axis=0
        ).astype(np.uint8)                           # [3, 8192]
        bcast = np.repeat(idxs, NTOT // 3, axis=0)   # [96, 8192]
        in_maps.append({"vt_t": vt_T, "bcast": bcast, "wt": wt})
    return in_maps


def kernel(vt, rgap, sgap, pcount, W, _trace=False, _tmpdir=None):
    nc = _get_nc()
    in_maps = _host_prep(vt, rgap, sgap, pcount, W)
    res = run_bass_kernel_spmd(
        nc, in_maps, list(range(N_CORES)),
        trace=_trace, **({"tmpdir": _tmpdir} if _tmpdir else {}),
    )
    full = np.empty((B, S, E + NTOT), dtype=np.float32)
    for m in range(N_CORES):
        sl = slice(m * BPC, (m + 1) * BPC)
        theta = np.asarray(res.results[m]["theta_t"]).astype(np.float32)
        ct = np.asarray(res.results[m]["ct_t"]).astype(np.float32)
        full[sl, :, :E] = theta.T.reshape(BPC, S, E)
        full[sl, :, E:] = ct.T.reshape(BPC, S, NTOT)
    if _trace:
        return full, res
    return full
